# revision 12
# baseline (speedup 1.0000x reference)
"""Trainium2 Bass kernel for masked bi-linear attention.

Computes, for full inputs
    k:    [B, KL, E] f32
    q:    [B, Q,  E] f32
    W:    [E, E]     f32
    mask: [B, Q, KL] i32 (0/1)
the reference
    qw    = q @ W                      [B, Q, E]
    s     = qw @ k^T                   [B, Q, KL]
    p     = softmax(s, axis=-1) * mask
    out   = p @ k                      [B, Q, E]

Sharding: data-parallel over B across 8 NeuronCores (2 batches/core),
W replicated. Each core runs the same Bass program on its B-slice.

Precision strategy: q/W/k cast to fp16 (11-bit mantissa) for the qw and
score matmuls -- single-pass LDWEIGHTS and transposes on the PE; softmax
in fp32 on ACT/DVE; probabilities (in [0,1]) in bf16 for p @ k.

Pipelining, per q-tile t:
  scores(t) [PE] -> row-max combine [DVE] -> pT transposes of tile t-1
  [PE] + copies [DVE] -> blockwise exp+mask (t) [ACT/DVE] -> PV matmuls
  of t-1 [PE] -> 1/z scale + output DMA.  Deferring PV by one tile keeps
  the PE on score matmuls while ACT/DVE run the softmax; emitting the
  pT copies before the mask multiplies keeps the (in-order) DVE queue
  from blocking the PV start.  Mask DMAs are prefetched at tile start
  and q DMAs one block ahead so neither sits behind the other in the
  sync HWDGE queue when needed.  knat is double-buffered so the deferred
  PV of a batch's last tile overlaps the next batch's k load.
"""

import numpy as np

import concourse.bacc as bacc
import concourse.mybir as mybir
import concourse.tile as tile
from concourse.bass_utils import run_bass_kernel_spmd
from concourse.masks import make_identity
from contextlib import ExitStack

dt = mybir.dt
AF = mybir.ActivationFunctionType
ALU = mybir.AluOpType
AX = mybir.AxisListType

P = 128

N_CORES = 8
B, Q_LEN, K_LEN, EMB = 16, 2048, 2048, 1024


def emit_attention(ctx, tc, k_ap, q_ap, w_ap, mask_ap, out_ap,
                   Bl, Q, KL, E, QB=512):
    """Emit the per-core attention program.

    k_ap [Bl, KL, E], q_ap [Bl, Q, E], w_ap [E, E], mask_ap [Bl, Q, KL],
    out_ap [Bl, Q, E].
    """
    nc = tc.nc
    f32, bf16, f16, i32 = dt.float32, dt.bfloat16, dt.float16, dt.int32

    assert Q % QB == 0 and QB % P == 0 and KL % P == 0 and E % P == 0
    EC = E // P          # e (contraction for qw) chunks
    KC = KL // P         # k chunks
    FC = E // P          # f chunks (qw output tiles)
    nqb = Q // QB
    qt_per_b = QB // P
    KB = min(512, KL)    # score psum block (<= 1 bank)
    nkb = KL // KB
    EB = min(512, E)     # PV psum block
    neb = E // EB
    GW = 4               # q/k transposes batched per psum bank
    PG = 4               # p transposes per group

    const = ctx.enter_context(tc.tile_pool(name="const", bufs=1))
    ident = const.tile([P, P], f32)
    make_identity(nc, ident[:])
    idb_t = const.tile([P, P], bf16)
    nc.gpsimd.tensor_copy(idb_t[:], ident[:])   # 0/1 exact in bf16
    idb = idb_t[:]
    idh_t = const.tile([P, P], f16)
    nc.gpsimd.tensor_copy(idh_t[:], ident[:])   # 0/1 exact in fp16
    idh = idh_t[:]

    big = ctx.enter_context(tc.tile_pool(name="big", bufs=1))
    qio = ctx.enter_context(tc.tile_pool(name="qio", bufs=5))
    q16p = ctx.enter_context(tc.tile_pool(name="q16p", bufs=3))
    mio = ctx.enter_context(tc.tile_pool(name="mio", bufs=2))
    ptp = ctx.enter_context(tc.tile_pool(name="ptp", bufs=4))
    work = ctx.enter_context(tc.tile_pool(name="work", bufs=2))
    small = ctx.enter_context(tc.tile_pool(name="small", bufs=3))
    psum = ctx.enter_context(tc.tile_pool(name="psum", bufs=4, space="PSUM"))
    psum_t = ctx.enter_context(tc.tile_pool(name="psum_t", bufs=2, space="PSUM"))
    psum_o = ctx.enter_context(tc.tile_pool(name="psum_o", bufs=1, space="PSUM"))

    # ---- W: loaded once per core as fp16; the DMA+cast emission happens
    # after the first q-block's loads so the kernel head starts on q
    # transposes instead of waiting for W
    wH = big.tile([P, EC * E], f16, tag="wH")

    def emit_w_load():
        for ec in range(EC):
            win = qio.tile([P, E], f32, tag="qin", name="win")
            nc.sync.dma_start(win[:], w_ap[ec * P:(ec + 1) * P, :])
            nc.scalar.copy(wH[:, ec * E:(ec + 1) * E], win[:])

    # deferred-PV state: (b, row0, spb, rz, knat)
    pending = []

    def pv_prep(st):
        # pT transposes (PE) + PSUM->SBUF copies (DVE) for the deferred
        # tile; emitted before the current tile's exp/mask so the DVE
        # queue serves the copies promptly
        spb = st[2]
        grps = []
        for g in range(KC // PG):
            pt = psum_t.tile([P, PG * P], bf16, tag="tp", name="pt")
            for j in range(PG):
                kc = g * PG + j
                nc.tensor.transpose(pt[:, j * P:(j + 1) * P],
                                    spb[:, kc * P:(kc + 1) * P], idb)
            ptsg = ptp.tile([P, PG * P], bf16, tag="pt", name="ptsg")
            nc.vector.tensor_copy(ptsg[:], pt[:])
            grps.append(ptsg)
        return grps

    def pv_mms(st, grps):
        b, row0, spb, rz, knat = st
        po = [psum_o.tile([P, EB], f32, tag=f"po{eh}", name=f"po{eh}")
              for eh in range(neb)]
        for g in range(KC // PG):
            ptsg = grps[g]
            for j in range(PG):
                kc = g * PG + j
                for eh in range(neb):
                    nc.tensor.matmul(
                        po[eh][:], ptsg[:, j * P:(j + 1) * P],
                        knat[:, kc * E + eh * EB: kc * E + (eh + 1) * EB],
                        start=(kc == 0), stop=(kc == KC - 1))
        for eh in range(neb):
            ot = mio.tile([P, EB], f32, tag="ot", name="ot")
            nc.scalar.activation(ot[:], po[eh][:], AF.Copy, scale=rz[:])
            nc.gpsimd.dma_start(
                out_ap[b, row0: row0 + P, eh * EB:(eh + 1) * EB], ot[:])

    def emit_pv(st):
        pv_mms(st, pv_prep(st))

    # ---- q loads: DMAs may be issued ahead (prefetched) of the
    # transpose emission so they don't queue behind mask DMAs
    def emit_qin_dmas(b, qb, qts):
        tiles = []
        for qt in qts:
            qin = qio.tile([P, E], f32, tag="qin", name="qin")
            nc.sync.dma_start(
                qin[:], q_ap[b, qb * QB + qt * P: qb * QB + (qt + 1) * P, :])
            tiles.append(qin)
        return tiles

    def emit_block_qT(b, qb, pre):
        qT = big.tile([P, EC, QB], f16, tag="qTh", name="qT")
        qins = pre + emit_qin_dmas(b, qb, range(len(pre), qt_per_b))
        for qt in range(qt_per_b):
            q16 = q16p.tile([P, E], f16, tag="q16", name="q16")
            nc.gpsimd.tensor_copy(q16[:], qins[qt][:])
            for eg in range(EC // GW):
                pt = psum_t.tile([P, GW * P], f16, tag="tp", name="pt")
                for j in range(GW):
                    ec = eg * GW + j
                    nc.tensor.transpose(
                        pt[:, j * P:(j + 1) * P],
                        q16[:, ec * P:(ec + 1) * P], idh)
                ptv = pt[:].rearrange("p (g c) -> p g c", g=GW)
                nc.scalar.copy(
                    qT[:, eg * GW:(eg + 1) * GW, qt * P:(qt + 1) * P], ptv)
        return qT

    def emit_block_qw(qT):
        qwT = big.tile([P, FC * QB], f16, tag="qwTh", name="qwT")
        for fc in range(FC):
            ps = psum.tile([P, QB], f32, tag="ps", name="ps")
            for ec in range(EC):
                nc.tensor.matmul(
                    ps[:], wH[:, ec * E + fc * P: ec * E + (fc + 1) * P],
                    qT[:, ec, :], start=(ec == 0), stop=(ec == EC - 1))
            nc.scalar.copy(qwT[:, fc * QB:(fc + 1) * QB], ps[:])
        return qwT

    def emit_k_phase(b):
        knat = big.tile([P, KC * E], bf16, tag="knat", name="knat", bufs=2)
        kTh = big.tile([P, EC, KL], f16, tag="kTh", name="kTh")

        def chunk(kc):
            kin = qio.tile([P, E], f32, tag="qin", name="kin")
            nc.sync.dma_start(kin[:], k_ap[b, kc * P:(kc + 1) * P, :])
            # bf16 rounding copy for the PV matmul rhs
            nc.vector.tensor_copy(knat[:, kc * E:(kc + 1) * E], kin[:])
            k16 = q16p.tile([P, E], f16, tag="q16", name="k16")
            nc.gpsimd.tensor_copy(k16[:], kin[:])
            for eg in range(EC // GW):
                pt = psum_t.tile([P, GW * P], f16, tag="tp", name="pt")
                for j in range(GW):
                    ec = eg * GW + j
                    nc.tensor.transpose(
                        pt[:, j * P:(j + 1) * P],
                        k16[:, ec * P:(ec + 1) * P], idh)
                ptv = pt[:].rearrange("p (g c) -> p g c", g=GW)
                nc.scalar.copy(
                    kTh[:, eg * GW:(eg + 1) * GW, kc * P:(kc + 1) * P], ptv)

        for kc in range(KC):
            chunk(kc)
        return knat, kTh

    for b in range(Bl):
        # first q-block prep runs before the K phase: its qw matmuls keep
        # the PE busy while the k DMA stream lands; knat is
        # double-buffered so the previous batch's deferred PV drains
        # during the k load
        qT = emit_block_qT(b, 0, [])
        if b == 0:
            emit_w_load()
        qwT = emit_block_qw(qT)
        knat, kTh = emit_k_phase(b)

        qin_pre = []
        for qb in range(nqb):
            if qb > 0:
                qT = emit_block_qT(b, qb, qin_pre)
                qin_pre = []
                qwT = emit_block_qw(qT)

            for qt in range(qt_per_b):
                row0 = qb * QB + qt * P
                # mask prefetch: in the sync queue before the next
                # block's q rows, consumed after this tile's exp
                mts = []
                for kb in range(nkb):
                    mt = mio.tile([P, KB], i32, tag="mask", name="mt",
                                  bufs=6)
                    nc.sync.dma_start(
                        mt[:], mask_ap[b, row0: row0 + P,
                                       kb * KB:(kb + 1) * KB])
                    mts.append(mt)

                sp = work.tile([P, KL], f32, tag="sp", name="sp", bufs=1)
                mx = small.tile([P, nkb], f32, tag="mx", name="mx")
                for kb in range(nkb):
                    ps_s = psum.tile([P, KB], f32, tag="ps", name="ps_s")
                    for fc in range(FC):
                        nc.tensor.matmul(
                            ps_s[:],
                            qwT[:, fc * QB + qt * P: fc * QB + (qt + 1) * P],
                            kTh[:, fc, kb * KB:(kb + 1) * KB],
                            start=(fc == 0), stop=(fc == FC - 1))
                    nc.scalar.copy(sp[:, kb * KB:(kb + 1) * KB], ps_s[:])
                    nc.vector.tensor_reduce(
                        mx[:, kb:kb + 1], sp[:, kb * KB:(kb + 1) * KB],
                        axis=AX.X, op=ALU.max)

                negm = small.tile([P, 1], f32, tag="negm", name="negm")
                nc.vector.tensor_reduce(negm[:], mx[:], axis=AX.X,
                                        op=ALU.max, negate=True)

                # transposes+copies of the deferred tile before this
                # tile's exp/mask occupy ACT+DVE
                grps = pv_prep(pending[0]) if pending else None

                spb = work.tile([P, KL], bf16, tag="spb", name="spb")
                zs = small.tile([P, nkb], f32, tag="zs", name="zs")
                for kb in range(nkb):
                    blk = slice(kb * KB, (kb + 1) * KB)
                    nc.scalar.activation(spb[:, blk], sp[:, blk], AF.Exp,
                                         bias=negm[:],
                                         accum_out=zs[:, kb:kb + 1])
                    nc.vector.scalar_tensor_tensor(
                        out=spb[:, blk], in0=mts[kb][:], scalar=1.0,
                        in1=spb[:, blk], op0=ALU.mult, op1=ALU.mult)
                z = small.tile([P, 1], f32, tag="z", name="z")
                nc.vector.tensor_reduce(z[:], zs[:], axis=AX.X, op=ALU.add)
                rz = small.tile([P, 1], f32, tag="rz", name="rz")
                nc.vector.reciprocal(rz[:], z[:])

                if pending:
                    pv_mms(pending.pop(0), grps)
                pending.append((b, row0, spb, rz, knat))

                # prefetch the next block's first q rows so they sit in
                # the sync queue ahead of the remaining mask DMAs
                if qt == qt_per_b - 3 and qb + 1 < nqb:
                    qin_pre = emit_qin_dmas(b, qb + 1, range(4))

    while pending:
        emit_pv(pending.pop(0))


def build_program(Bl, Q, KL, E, QB=512):
    nc = bacc.Bacc("TRN2", target_bir_lowering=False, debug=False)
    k_t = nc.dram_tensor("k", [Bl, KL, E], dt.float32, kind="ExternalInput")
    q_t = nc.dram_tensor("q", [Bl, Q, E], dt.float32, kind="ExternalInput")
    w_t = nc.dram_tensor("W", [E, E], dt.float32, kind="ExternalInput")
    m_t = nc.dram_tensor("mask", [Bl, Q, KL], dt.int32, kind="ExternalInput")
    o_t = nc.dram_tensor("out", [Bl, Q, E], dt.float32, kind="ExternalOutput")
    with tile.TileContext(nc) as tc:
        with ExitStack() as ctx:
            emit_attention(ctx, tc, k_t.ap(), q_t.ap(), w_t.ap(), m_t.ap(),
                           o_t.ap(), Bl, Q, KL, E, QB=QB)
    nc.compile()
    return nc


def kernel(k: np.ndarray, q: np.ndarray, W: np.ndarray, mask: np.ndarray,
           **run_kwargs) -> np.ndarray:
    assert k.shape == (B, K_LEN, EMB) and q.shape == (B, Q_LEN, EMB)
    assert W.shape == (EMB, EMB) and mask.shape == (B, Q_LEN, K_LEN)
    Bl = B // N_CORES
    nc = build_program(Bl, Q_LEN, K_LEN, EMB)
    in_maps = []
    for c in range(N_CORES):
        sl = slice(c * Bl, (c + 1) * Bl)
        in_maps.append({
            "k": np.ascontiguousarray(k[sl], dtype=np.float32),
            "q": np.ascontiguousarray(q[sl], dtype=np.float32),
            "W": np.ascontiguousarray(W, dtype=np.float32),
            "mask": np.ascontiguousarray(mask[sl], dtype=np.int32),
        })
    res = run_bass_kernel_spmd(nc, in_maps, core_ids=list(range(N_CORES)),
                               **run_kwargs)
    out = np.concatenate([r["out"] for r in res.results], axis=0)
    if run_kwargs.get("trace"):
        kernel.last_exec_time_ns = res.exec_time_ns
        kernel.last_result = res
    return out


kernel.last_exec_time_ns = None
kernel.last_result = None


# revision 13
# speedup vs baseline: 1.3188x; 1.3188x over previous
"""Trainium2 Bass kernel for masked bi-linear attention.

Computes, for full inputs
    k:    [B, KL, E] f32
    q:    [B, Q,  E] f32
    W:    [E, E]     f32
    mask: [B, Q, KL] i32 (0/1)
the reference
    qw    = q @ W                      [B, Q, E]
    s     = qw @ k^T                   [B, Q, KL]
    p     = softmax(s, axis=-1) * mask
    out   = p @ k                      [B, Q, E]

Sharding: data-parallel over B across 8 NeuronCores (2 batches/core),
W replicated. Each core runs the same Bass program on its B-slice.

Precision strategy: q/W/k cast to fp16 (11-bit mantissa) for the qw and
score matmuls -- single-pass LDWEIGHTS and transposes on the PE; softmax
in fp32 on ACT/DVE; probabilities (in [0,1]) in bf16 for p @ k.

Pipelining, per q-tile t:
  scores(t) [PE] -> row-max combine [DVE] -> pT transposes of tile t-1
  [PE] + copies [DVE] -> blockwise exp+mask (t) [ACT/DVE] -> PV matmuls
  of t-1 [PE] -> 1/z scale + output DMA.  Deferring PV by one tile keeps
  the PE on score matmuls while ACT/DVE run the softmax; emitting the
  pT copies before the mask multiplies keeps the (in-order) DVE queue
  from blocking the PV start.  Mask DMAs are prefetched at tile start
  and q DMAs one block ahead so neither sits behind the other in the
  sync HWDGE queue when needed.  knat is double-buffered so the deferred
  PV of a batch's last tile overlaps the next batch's k load.
"""

import numpy as np

import concourse.bacc as bacc
import concourse.mybir as mybir
import concourse.tile as tile
from concourse.bass_utils import run_bass_kernel_spmd
from concourse.masks import make_identity
from contextlib import ExitStack

dt = mybir.dt
AF = mybir.ActivationFunctionType
ALU = mybir.AluOpType
AX = mybir.AxisListType

P = 128

N_CORES = 8
B, Q_LEN, K_LEN, EMB = 16, 2048, 2048, 1024


def emit_attention(ctx, tc, k_ap, q_ap, w_ap, mask_ap, out_ap,
                   Bl, Q, KL, E, QB=512):
    """Emit the per-core attention program.

    k_ap [Bl, KL, E], q_ap [Bl, Q, E], w_ap [E, E], mask_ap [Bl, Q, KL],
    out_ap [Bl, Q, E].
    """
    nc = tc.nc
    f32, bf16, f16, i32 = dt.float32, dt.bfloat16, dt.float16, dt.int32

    assert Q % QB == 0 and QB % P == 0 and KL % P == 0 and E % P == 0
    EC = E // P          # e (contraction for qw) chunks
    KC = KL // P         # k chunks
    FC = E // P          # f chunks (qw output tiles)
    nqb = Q // QB
    qt_per_b = QB // P
    KB = min(512, KL)    # score psum block (<= 1 bank)
    nkb = KL // KB
    EB = min(512, E)     # PV psum block
    neb = E // EB
    GW = 4               # q/k transposes batched per psum bank
    PG = 4               # p transposes per group

    const = ctx.enter_context(tc.tile_pool(name="const", bufs=1))
    ident = const.tile([P, P], f32)
    make_identity(nc, ident[:])
    idb_t = const.tile([P, P], bf16)
    nc.gpsimd.tensor_copy(idb_t[:], ident[:])   # 0/1 exact in bf16
    idb = idb_t[:]
    idh_t = const.tile([P, P], f16)
    nc.gpsimd.tensor_copy(idh_t[:], ident[:])   # 0/1 exact in fp16
    idh = idh_t[:]

    big = ctx.enter_context(tc.tile_pool(name="big", bufs=1))
    qio = ctx.enter_context(tc.tile_pool(name="qio", bufs=5))
    q16p = ctx.enter_context(tc.tile_pool(name="q16p", bufs=3))
    mio = ctx.enter_context(tc.tile_pool(name="mio", bufs=2))
    ptp = ctx.enter_context(tc.tile_pool(name="ptp", bufs=4))
    work = ctx.enter_context(tc.tile_pool(name="work", bufs=2))
    small = ctx.enter_context(tc.tile_pool(name="small", bufs=3))
    psum = ctx.enter_context(tc.tile_pool(name="psum", bufs=4, space="PSUM"))
    psum_t = ctx.enter_context(tc.tile_pool(name="psum_t", bufs=2, space="PSUM"))
    psum_o = ctx.enter_context(tc.tile_pool(name="psum_o", bufs=1, space="PSUM"))

    # ---- W: loaded once per core as fp16; the DMA+cast emission happens
    # after the first q-block's loads so the kernel head starts on q
    # transposes instead of waiting for W
    wH = big.tile([P, EC * E], f16, tag="wH")

    def emit_w_load():
        for ec in range(EC):
            win = qio.tile([P, E], f32, tag="qin", name="win")
            nc.sync.dma_start(win[:], w_ap[ec * P:(ec + 1) * P, :])
            nc.scalar.copy(wH[:, ec * E:(ec + 1) * E], win[:])

    # deferred-PV state: (b, row0, spb, rz, knat)
    pending = []

    def pv_prep(st):
        # pT transposes (PE) + PSUM->SBUF copies (DVE) for the deferred
        # tile; emitted before the current tile's exp/mask so the DVE
        # queue serves the copies promptly
        spb = st[2]
        grps = []
        for g in range(KC // PG):
            pt = psum_t.tile([P, PG * P], bf16, tag="tp", name="pt")
            for j in range(PG):
                kc = g * PG + j
                nc.tensor.transpose(pt[:, j * P:(j + 1) * P],
                                    spb[:, kc * P:(kc + 1) * P], idb)
            ptsg = ptp.tile([P, PG * P], bf16, tag="pt", name="ptsg")
            nc.vector.tensor_copy(ptsg[:], pt[:])
            grps.append(ptsg)
        return grps

    def pv_mms(st, grps):
        b, row0, spb, rz, knat = st
        po = [psum_o.tile([P, EB], f32, tag=f"po{eh}", name=f"po{eh}")
              for eh in range(neb)]
        for g in range(KC // PG):
            ptsg = grps[g]
            for j in range(PG):
                kc = g * PG + j
                for eh in range(neb):
                    nc.tensor.matmul(
                        po[eh][:], ptsg[:, j * P:(j + 1) * P],
                        knat[:, kc * E + eh * EB: kc * E + (eh + 1) * EB],
                        start=(kc == 0), stop=(kc == KC - 1))
        for eh in range(neb):
            ot = mio.tile([P, EB], f32, tag="ot", name="ot")
            nc.scalar.activation(ot[:], po[eh][:], AF.Copy, scale=rz[:])
            nc.gpsimd.dma_start(
                out_ap[b, row0: row0 + P, eh * EB:(eh + 1) * EB], ot[:])

    def emit_pv(st):
        pv_mms(st, pv_prep(st))

    # ---- q loads: DMAs may be issued ahead (prefetched) of the
    # transpose emission so they don't queue behind mask DMAs
    def emit_qin_dmas(b, qb, qts):
        tiles = []
        for qt in qts:
            qin = qio.tile([P, E], f32, tag="qin", name="qin")
            nc.sync.dma_start(
                qin[:], q_ap[b, qb * QB + qt * P: qb * QB + (qt + 1) * P, :])
            tiles.append(qin)
        return tiles

    def emit_block_qT(b, qb, pre):
        qT = big.tile([P, EC, QB], f16, tag="qTh", name="qT")
        qins = pre + emit_qin_dmas(b, qb, range(len(pre), qt_per_b))
        for qt in range(qt_per_b):
            q16 = q16p.tile([P, E], f16, tag="q16", name="q16")
            nc.vector.tensor_copy(q16[:], qins[qt][:])
            for eg in range(EC // GW):
                pt = psum_t.tile([P, GW * P], f16, tag="tp", name="pt")
                for j in range(GW):
                    ec = eg * GW + j
                    nc.tensor.transpose(
                        pt[:, j * P:(j + 1) * P],
                        q16[:, ec * P:(ec + 1) * P], idh)
                ptv = pt[:].rearrange("p (g c) -> p g c", g=GW)
                nc.scalar.copy(
                    qT[:, eg * GW:(eg + 1) * GW, qt * P:(qt + 1) * P], ptv)
        return qT

    def emit_block_qw(qT):
        qwT = big.tile([P, FC * QB], f16, tag="qwTh", name="qwT")
        for fc in range(FC):
            ps = psum.tile([P, QB], f32, tag="ps", name="ps")
            for ec in range(EC):
                nc.tensor.matmul(
                    ps[:], wH[:, ec * E + fc * P: ec * E + (fc + 1) * P],
                    qT[:, ec, :], start=(ec == 0), stop=(ec == EC - 1))
            nc.scalar.copy(qwT[:, fc * QB:(fc + 1) * QB], ps[:])
        return qwT

    def emit_k_phase(b):
        knat = big.tile([P, KC * E], bf16, tag="knat", name="knat", bufs=2)
        kTh = big.tile([P, EC, KL], f16, tag="kTh", name="kTh")

        def chunk(kc):
            kin = qio.tile([P, E], f32, tag="qin", name="kin")
            nc.sync.dma_start(kin[:], k_ap[b, kc * P:(kc + 1) * P, :])
            # bf16 rounding copy for the PV matmul rhs
            nc.vector.tensor_copy(knat[:, kc * E:(kc + 1) * E], kin[:])
            k16 = q16p.tile([P, E], f16, tag="q16", name="k16")
            nc.vector.tensor_copy(k16[:], kin[:])
            for eg in range(EC // GW):
                pt = psum_t.tile([P, GW * P], f16, tag="tp", name="pt")
                for j in range(GW):
                    ec = eg * GW + j
                    nc.tensor.transpose(
                        pt[:, j * P:(j + 1) * P],
                        k16[:, ec * P:(ec + 1) * P], idh)
                ptv = pt[:].rearrange("p (g c) -> p g c", g=GW)
                nc.scalar.copy(
                    kTh[:, eg * GW:(eg + 1) * GW, kc * P:(kc + 1) * P], ptv)

        for kc in range(KC):
            chunk(kc)
        return knat, kTh

    for b in range(Bl):
        # first q-block prep runs before the K phase: its qw matmuls keep
        # the PE busy while the k DMA stream lands; knat is
        # double-buffered so the previous batch's deferred PV drains
        # during the k load
        qT = emit_block_qT(b, 0, [])
        if b == 0:
            emit_w_load()
        qwT = emit_block_qw(qT)
        knat, kTh = emit_k_phase(b)

        qin_pre = []
        for qb in range(nqb):
            if qb > 0:
                qT = emit_block_qT(b, qb, qin_pre)
                qin_pre = []
                qwT = emit_block_qw(qT)

            for qt in range(qt_per_b):
                row0 = qb * QB + qt * P
                # mask prefetch: in the sync queue before the next
                # block's q rows, consumed after this tile's exp
                mts = []
                for kb in range(nkb):
                    mt = mio.tile([P, KB], i32, tag="mask", name="mt",
                                  bufs=6)
                    nc.sync.dma_start(
                        mt[:], mask_ap[b, row0: row0 + P,
                                       kb * KB:(kb + 1) * KB])
                    mts.append(mt)

                sp = work.tile([P, KL], f32, tag="sp", name="sp", bufs=1)
                mx = small.tile([P, nkb], f32, tag="mx", name="mx")
                for kb in range(nkb):
                    ps_s = psum.tile([P, KB], f32, tag="ps", name="ps_s")
                    for fc in range(FC):
                        nc.tensor.matmul(
                            ps_s[:],
                            qwT[:, fc * QB + qt * P: fc * QB + (qt + 1) * P],
                            kTh[:, fc, kb * KB:(kb + 1) * KB],
                            start=(fc == 0), stop=(fc == FC - 1))
                    nc.scalar.copy(sp[:, kb * KB:(kb + 1) * KB], ps_s[:])
                    nc.vector.tensor_reduce(
                        mx[:, kb:kb + 1], sp[:, kb * KB:(kb + 1) * KB],
                        axis=AX.X, op=ALU.max)

                negm = small.tile([P, 1], f32, tag="negm", name="negm")
                nc.vector.tensor_reduce(negm[:], mx[:], axis=AX.X,
                                        op=ALU.max, negate=True)

                # transposes+copies of the deferred tile before this
                # tile's exp/mask occupy ACT+DVE
                grps = pv_prep(pending[0]) if pending else None

                spb = work.tile([P, KL], bf16, tag="spb", name="spb")
                zs = small.tile([P, nkb], f32, tag="zs", name="zs")
                for kb in range(nkb):
                    blk = slice(kb * KB, (kb + 1) * KB)
                    nc.scalar.activation(spb[:, blk], sp[:, blk], AF.Exp,
                                         bias=negm[:],
                                         accum_out=zs[:, kb:kb + 1])
                    nc.vector.scalar_tensor_tensor(
                        out=spb[:, blk], in0=mts[kb][:], scalar=1.0,
                        in1=spb[:, blk], op0=ALU.mult, op1=ALU.mult)
                z = small.tile([P, 1], f32, tag="z", name="z")
                nc.vector.tensor_reduce(z[:], zs[:], axis=AX.X, op=ALU.add)
                rz = small.tile([P, 1], f32, tag="rz", name="rz")
                nc.vector.reciprocal(rz[:], z[:])

                if pending:
                    pv_mms(pending.pop(0), grps)
                pending.append((b, row0, spb, rz, knat))

                # prefetch the next block's first q rows so they sit in
                # the sync queue ahead of the remaining mask DMAs
                if qt == qt_per_b - 3 and qb + 1 < nqb:
                    qin_pre = emit_qin_dmas(b, qb + 1, range(4))

    while pending:
        emit_pv(pending.pop(0))


def build_program(Bl, Q, KL, E, QB=512):
    nc = bacc.Bacc("TRN2", target_bir_lowering=False, debug=False)
    k_t = nc.dram_tensor("k", [Bl, KL, E], dt.float32, kind="ExternalInput")
    q_t = nc.dram_tensor("q", [Bl, Q, E], dt.float32, kind="ExternalInput")
    w_t = nc.dram_tensor("W", [E, E], dt.float32, kind="ExternalInput")
    m_t = nc.dram_tensor("mask", [Bl, Q, KL], dt.int32, kind="ExternalInput")
    o_t = nc.dram_tensor("out", [Bl, Q, E], dt.float32, kind="ExternalOutput")
    with tile.TileContext(nc) as tc:
        with ExitStack() as ctx:
            emit_attention(ctx, tc, k_t.ap(), q_t.ap(), w_t.ap(), m_t.ap(),
                           o_t.ap(), Bl, Q, KL, E, QB=QB)
    nc.compile()
    return nc


def kernel(k: np.ndarray, q: np.ndarray, W: np.ndarray, mask: np.ndarray,
           **run_kwargs) -> np.ndarray:
    assert k.shape == (B, K_LEN, EMB) and q.shape == (B, Q_LEN, EMB)
    assert W.shape == (EMB, EMB) and mask.shape == (B, Q_LEN, K_LEN)
    Bl = B // N_CORES
    nc = build_program(Bl, Q_LEN, K_LEN, EMB)
    in_maps = []
    for c in range(N_CORES):
        sl = slice(c * Bl, (c + 1) * Bl)
        in_maps.append({
            "k": np.ascontiguousarray(k[sl], dtype=np.float32),
            "q": np.ascontiguousarray(q[sl], dtype=np.float32),
            "W": np.ascontiguousarray(W, dtype=np.float32),
            "mask": np.ascontiguousarray(mask[sl], dtype=np.int32),
        })
    res = run_bass_kernel_spmd(nc, in_maps, core_ids=list(range(N_CORES)),
                               **run_kwargs)
    out = np.concatenate([r["out"] for r in res.results], axis=0)
    if run_kwargs.get("trace"):
        kernel.last_exec_time_ns = res.exec_time_ns
        kernel.last_result = res
    return out


kernel.last_exec_time_ns = None
kernel.last_result = None
